# revision 59
# baseline (speedup 1.0000x reference)
"""Trainium2 Bass kernel for the AdaptiveGaussKronrod VJP quadrature problem.

Math (reference, flattened over N = S*15 = 1920 quadrature nodes):
    phi = sin(t (x) freqs)                  [N, D]
    Z   = phi @ W + b                       [N, D]
    G   = (h*wk)_n * cos(t (x) afreqs) * (1 - tanh(Z)^2)
    out = phi^T @ G                         [D, D]

Key algebraic optimizations:
  * sin(t*f) and cos(t*f) on t in [0,1], f in [0.5,3] are analytic kernels
    with exponentially decaying singular values — rank 6 reproduces both to
    ~1e-8 relative. phi = U V^T and cos = Uc Vc^T with the t-side factors
    fixed (precomputed SVD) and the f-side factors evaluated on the host
    via Chebyshev fits of the right singular functions. This collapses the
    two 16-GFLOP GEMMs into rank-6 contractions:
        A   = V^T @ W            [6, J]   (the only pass over W)
        Z   = [U|1] @ [A;b]      [N, J]   (bias via appended ones row)
        cos = Uc @ Vc^T          [N, J]   (PE matmul, not ScalarE Sin)
    and with hw folded into U2 = U*hw and B distributed as
        B   = (+U2)^T cos + (-U2)^T (cos*y*y)     y = tanh(Z)
    the elementwise work is one ScalarE Tanh plus two chained bf16
    tensor-tensor multiplies per tile:
        u = cos*y ; v = u*y
    All matmuls bf16 with fp32 PSUM accumulation; fro rel err ~4e-3
    (gate 2e-2).
  * out = V @ B [D, J] per core, cast bf16, DMA'd out.

Sharding: output-column parallel over 8 cores (J = D/8 = 512 columns).
No collectives; host concatenates.

Schedule notes: the Sync queue issues each DMA descriptor in ~0.7us, so
all small constants ride in TWO packed DMAs ahead of the four 1MB W
chunks; the PE warms on dummy matmuls then generates cos during the W
stream (HAM stays at K=8/8); ScalarE only does Tanh + PSUM->bf16 copies;
DVE ops run on [P, 3*J] triples to amortize the per-op bubble.
"""

import math

import numpy as np

D = 4096
S = 128
NCORES = 8
J = D // NCORES     # output columns per core (512)
N = S * 15          # 1920 quadrature nodes
P = 128
KT = D // P         # 32 k-tiles over D
MT = N // P         # 15 m-tiles over N
OT = D // P         # 32 output row tiles
R = 6               # separable rank of sin(t*f) / cos(t*f)
KZ = R + 1          # rank rows + ones row (bias)
WCH = 4             # W DMA chunks
# uneven split (k-tiles per chunk): a small last chunk so the A-GEMM tail
# after the final DMA lands is ~2 matmuls, not 8
WSPLIT = [10, 10, 10, 2]
WOFF = [0, 10, 20, 30]  # k-tile offsets

# pack layout (partition dim 7): columns of the bf16 "pack" tensor
PK_UTZ = 0                  # [7, N]    U^T rows + ones row
PK_VTO = PK_UTZ + N         # [6, D]    V^T for out-GEMM
PK_UCO = PK_VTO + D         # [6, N]    Ucos^T for cos-GEMM
PK_VCA = PK_UCO + N         # [6, J]    Vc(afreqs shard)^T      (per-core)
PK_BRW = PK_VCA + J         # [1, J] at row 6: bias row          (per-core)
PK_W = PK_BRW + J           # total pack width

_NODES_NEG = np.array([-0.9914553711208126, -0.9491079123427585, -0.8648644233597691,
                       -0.7415311855993945, -0.5860872354676911, -0.4058451513773972,
                       -0.20778495500789848, 0.0])
_WK_HALF = np.array([0.022935322010529224, 0.06309209262997856, 0.10479001032225019,
                     0.14065325971552592, 0.1690047266392679, 0.19035057806478542,
                     0.20443294007529889, 0.20948214108472782])
GK_NODES = np.concatenate([-_NODES_NEG[:-1][::-1], _NODES_NEG])  # [15]
GK_WK = np.concatenate([_WK_HALF[:-1][::-1], _WK_HALF])          # [15]

_FDOM = (0.45, 3.05)    # freq domain covered by the separable bases


def _host_constants():
    edges = np.linspace(0.0, 1.0, S + 1)
    a_s, b_s = edges[:-1], edges[1:]
    h = (b_s - a_s) / 2.0
    c = (a_s + b_s) / 2.0
    t = (c[:, None] + h[:, None] * GK_NODES[None, :]).reshape(-1)
    hw = (h[:, None] * GK_WK[None, :]).reshape(-1)
    return t, hw  # float64 [N]


_FCACHE = {}


def _factorization():
    """Rank-R separable bases: U/Uc on the exact t nodes + Chebyshev fits
    of the R right singular functions, for sin(t f) and cos(t f)."""
    if "f" in _FCACHE:
        return _FCACHE["f"]
    t, hw = _host_constants()
    fgrid = np.linspace(_FDOM[0], _FDOM[1], 1200)

    def fac(fn):
        M = fn(np.outer(t, fgrid))
        Us, ss, Vt = np.linalg.svd(M, full_matrices=False)
        U = Us[:, :R] * ss[:R]
        chebs = [np.polynomial.chebyshev.Chebyshev.fit(
            fgrid, Vt[k], 24, domain=list(_FDOM)) for k in range(R)]
        return U, chebs

    Usin, chs = fac(np.sin)
    Ucos, chc = fac(np.cos)
    _FCACHE["f"] = (t, hw, Usin, chs, Ucos, chc)
    return _FCACHE["f"]


def _eval_chebs(chebs, f):
    f = np.clip(f.astype(np.float64), _FDOM[0], _FDOM[1])
    return np.stack([ck(f) for ck in chebs], axis=1)  # [len(f), R] float64


def _patch_act_tables():
    """Force Sin AND Tanh to resolve to one table set so the act-table-load
    pass emits a single load (Copy lives in every set)."""
    import concourse.bacc as bacc_mod
    from concourse import mybir

    if getattr(bacc_mod, "_act_tables_pinned", False):
        return
    orig = bacc_mod.get_activation_tables
    Sin = mybir.ActivationFunctionType.Sin
    Tanh = mybir.ActivationFunctionType.Tanh

    def patched(arch):
        tabs = orig(arch)
        out = {}
        for name, funcs in tabs.items():
            if (Sin in funcs) and (Tanh in funcs):
                out[name] = funcs
            else:
                out[name] = funcs - {Sin, Tanh}
        return out

    bacc_mod.get_activation_tables = patched
    bacc_mod._act_tables_pinned = True


def build_bass():
    """Build and compile the per-core Bass graph (identical on all 8 cores)."""
    from contextlib import ExitStack

    import concourse.bass as bass
    import concourse.tile as tile
    from concourse import bacc, mybir

    _patch_act_tables()

    f32 = mybir.dt.float32
    bf16 = mybir.dt.bfloat16
    Tanh = mybir.ActivationFunctionType.Tanh
    Copy = mybir.ActivationFunctionType.Copy

    nc = bacc.Bacc("TRN2", target_bir_lowering=False, debug=False,
                   enable_asserts=False)

    wkt_ext = nc.dram_tensor("wkt", [P, KT * J], bf16, kind="ExternalInput")
    pack_ext = nc.dram_tensor("pack", [KZ, PK_W], bf16, kind="ExternalInput")
    cstb_ext = nc.dram_tensor("cstb", [P, (KT + MT * 2) * R], bf16,
                              kind="ExternalInput")
    # V^T / [U|1]^T replicated at partition offsets 0/32/64/96 for 4-way
    # row-group packed matmuls (K=6/7 uses one 32-row strip of the array)
    vto4_ext = nc.dram_tensor("vto4", [3 * 32 + R, D], bf16,
                              kind="ExternalInput")
    out_ext = nc.dram_tensor("out", [P, OT * J], bf16, kind="ExternalOutput")

    TRP = 5                  # 5 triples cover the 15 m-tiles
    with tile.TileContext(nc) as tc, ExitStack() as ctx:
        consts = ctx.enter_context(tc.tile_pool(name="consts", bufs=1))
        wp = ctx.enter_context(tc.tile_pool(name="wp", bufs=1))
        yp = ctx.enter_context(tc.tile_pool(name="y", bufs=2))
        up = ctx.enter_context(tc.tile_pool(name="u", bufs=2))
        vp = ctx.enter_context(tc.tile_pool(name="v", bufs=2))
        stg = ctx.enter_context(tc.tile_pool(name="stg", bufs=3))
        aps = ctx.enter_context(
            tc.tile_pool(name="aps", bufs=1, space=bass.MemorySpace.PSUM))
        bps = ctx.enter_context(
            tc.tile_pool(name="bps", bufs=1, space=bass.MemorySpace.PSUM))
        # shared by warmup / cos / Z-pairs / out; Z-pairs are [P, 2J] so
        # each of the 3 buffers spans 2 PSUM banks (6 banks + aps + bps = 8)
        ops = ctx.enter_context(
            tc.tile_pool(name="ops", bufs=3, space=bass.MemorySpace.PSUM))

        # ---- PE warm-up: >=3.4us of contiguous dummy matmuls so HAM hits
        # K=8/8; the cos/A streams then keep it warm ----
        dummy = consts.tile([P, 256], bf16, tag="dummy")
        nc.vector.memset(dummy[:], 0.0)
        wps = ops.tile([P, J], f32, tag="opsum", name="warmps")
        for i in range(36):
            nc.tensor.matmul(wps[:, 0:128], lhsT=dummy[:, 0:128],
                             rhs=dummy[:, 128:256], start=True, stop=True)

        # ---- packed consts first (tiny, and the DMA queue is FIFO — they
        # must not trail the 11us W stream), then the W chunks ----
        cstb = consts.tile([P, (KT + MT * 2) * R], bf16, tag="cstb")
        nc.sync.dma_start(cstb[:], cstb_ext[:])
        vkt = cstb[:, 0:KT * R]
        u2t = cstb[:, KT * R:(KT + MT * 2) * R]
        pack = consts.tile([KZ, PK_W], bf16, tag="pack")
        nc.sync.dma_start(pack[:], pack_ext[:])
        utz = pack[0:KZ, PK_UTZ:PK_UTZ + N]
        vto = pack[0:R, PK_VTO:PK_VTO + D]
        ucot = pack[0:R, PK_UCO:PK_UCO + N]
        vca = pack[0:R, PK_VCA:PK_VCA + J]
        wt = []
        for c in range(WCH):
            w = wp.tile([P, WSPLIT[c] * J], bf16, tag=f"w{c}", name=f"w{c}")
            nc.sync.dma_start(w[:], wkt_ext[:, WOFF[c] * J:(WOFF[c] + WSPLIT[c]) * J])
            wt.append(w)
        # replicated V^T lands right after the W stream, well before the
        # out phase needs it
        vto4 = consts.tile([3 * 32 + R, D], bf16, tag="vto4")
        nc.sync.dma_start(vto4[:], vto4_ext[:])

        # act-table trigger: pulls the single ACT table load to kernel start
        zero1 = consts.tile([1, 1], f32, tag="zero1")
        nc.vector.memset(zero1[:], 0.0)
        scr1 = consts.tile([1, 1], f32, tag="scr1")
        nc.scalar.activation(scr1[:], zero1[:], Tanh, bias=0.0)

        # ---- cos = Uc @ Vc^T on the PE (paired PSUM banks -> ScalarE
        # copies) interleaved with A = V^T W (DMA-paced): the PE queue is
        # in-order, so A-matmuls must not sit behind the whole cos stream ----
        cosall = consts.tile([P, MT * J], bf16, tag="cosall")
        apsum = aps.tile([R, J], f32, tag="apsum")

        def emit_cos_pair(pr):
            for half in range(2):
                m = 2 * pr + half
                if m >= MT:
                    continue
                cps = ops.tile([P, J], f32, tag="opsum", name=f"cps{m}")
                nc.tensor.matmul(cps[:],
                                 lhsT=ucot[:, m * P:(m + 1) * P],
                                 rhs=vca[:], start=True, stop=True)
                nc.scalar.activation(cosall[:, m * J:(m + 1) * J],
                                     cps[:], Copy)

        def emit_a_chunk(c):
            for kk in range(WSPLIT[c]):
                k = WOFF[c] + kk
                nc.tensor.matmul(apsum[:],
                                 lhsT=vkt[:, k * R:(k + 1) * R],
                                 rhs=wt[c][:, kk * J:(kk + 1) * J],
                                 start=(k == 0), stop=(k == KT - 1))

        bpsum = bps.tile([R, J], f32, tag="bpsum")

        def emit_bc(ms):
            # B cos-term: B += (+U2)^T cos (needs only the cos tiles)
            for m in ms:
                nc.tensor.matmul(bpsum[:],
                                 lhsT=u2t[:, 2 * m * R:(2 * m + 1) * R],
                                 rhs=cosall[:, m * J:(m + 1) * J],
                                 start=(m == 0), stop=False)

        emit_cos_pair(0)
        emit_cos_pair(1)
        emit_a_chunk(0)
        emit_cos_pair(2)
        emit_cos_pair(3)
        emit_a_chunk(1)
        emit_cos_pair(4)
        emit_cos_pair(5)
        emit_cos_pair(6)
        emit_a_chunk(2)
        emit_cos_pair(7)
        emit_bc(range(MT))      # fills the PE while the last W chunk lands
        emit_a_chunk(3)

        # ---- second warm-up burst: back-to-back dummy matmuls after the
        # last A matmul so HAM is at K=8/8 for the whole middle phase
        # (the A->bf16 cast proceeds on DVE underneath it) ----
        bw = ops.tile([P, J], f32, tag="opsum", name="warm2")
        for i in range(40):
            nc.tensor.matmul(bw[:, 0:128], lhsT=dummy[:, 0:128],
                             rhs=dummy[:, 128:256], start=True, stop=True)

        # A psum -> bf16 rhs rows, replicated at the 4 row-group offsets;
        # bias rides in rows 32g+6 (pack brow rows 0..5 are zero and the A
        # cast overwrites them)
        asb4 = consts.tile([KZ, J], bf16, tag="asb4")
        nc.vector.tensor_copy(asb4[:], pack[0:KZ, PK_BRW:PK_BRW + J])
        nc.vector.tensor_copy(asb4[0:R, :], apsum[:])

        # PE HAM keep-alive: short back-to-back dummy bursts into the
        # (dead after A-cast) aps bank — consecutive matmuls to one tile
        # carry no semaphores, so these fill PE gaps at ~56ns each and
        # stop the activity monitor from re-throttling the clock
        def miniburst(n):
            dz = aps.tile([P, 64], f32, tag="apsum", name="ka")
            for _ in range(n):
                nc.tensor.matmul(dz[:], lhsT=dummy[:, 0:128],
                                 rhs=dummy[:, 128:192], start=True, stop=True)

        # ---- middle: Z-pair matmuls into double-bank PSUM, ONE wide
        # tanh per pair (ScalarE reads up to 4K PSUM elements per op —
        # 8 activations instead of 15), y in a single big tile so the
        # DVE triples slice it freely; u=cos*y, v=u*y -> -U2 B accum ----
        yall = consts.tile([P, MT * J], bf16, tag="yall")

        def emit_triple(tr):
            miniburst(8)
            u = up.tile([P, 3 * J], bf16, tag="u", name=f"u{tr}")
            nc.vector.tensor_mul(u[:], cosall[:, 3 * tr * J:(3 * tr + 3) * J],
                                 yall[:, 3 * tr * J:(3 * tr + 3) * J])
            v = vp.tile([P, 3 * J], bf16, tag="v", name=f"v{tr}")
            nc.vector.tensor_mul(v[:], u[:],
                                 yall[:, 3 * tr * J:(3 * tr + 3) * J])
            for third in range(3):
                m = 3 * tr + third
                nc.tensor.matmul(bpsum[:],
                                 lhsT=u2t[:, (2 * m + 1) * R:(2 * m + 2) * R],
                                 rhs=v[:, third * J:(third + 1) * J],
                                 start=False, stop=(m == MT - 1))

        ready = {1: 0, 2: 1, 4: 2, 5: 3, 7: 4}   # pair idx -> triple ready
        for pr in range(8):
            width = 2 * J if 2 * pr + 1 < MT else J
            zp = ops.tile([P, width], f32, tag="opsum", name=f"zp{pr}")
            for half in range(width // J):
                m = 2 * pr + half
                nc.tensor.matmul(zp[:, half * J:(half + 1) * J],
                                 lhsT=utz[:, m * P:(m + 1) * P],
                                 rhs=asb4[0:KZ, :], start=True, stop=True)
            nc.scalar.activation(yall[:, 2 * pr * J:2 * pr * J + width],
                                 zp[:, 0:width], Tanh, bias=0.0)
            if pr in ready:
                emit_triple(ready[pr])
        # B replicated at partition offsets 0/32/64/96 (ScalarE — DVE is
        # still draining the last v-multiply at this boundary)
        bsb4 = consts.tile([3 * 32 + R, J], bf16, tag="bsb4")
        # alternate engines: DVE is free right after the last v-multiply,
        # ScalarE is still draining the last tanh — the g=0 copy gates the
        # first out matmul, so it goes to DVE
        nc.vector.tensor_copy(bsb4[0:R, :], bpsum[:])
        nc.scalar.activation(bsb4[32:32 + R, :], bpsum[:], Copy)
        nc.vector.tensor_copy(bsb4[64:64 + R, :], bpsum[:])
        nc.scalar.activation(bsb4[96:96 + R, :], bpsum[:], Copy)

        # ---- out = V @ B: 4-way row-group-packed matmuls, each into its
        # own PSUM bank (no inter-matmul deps, so the PE overlaps the row
        # groups), per-tile casts alternating ScalarE/DVE, staged 512KB
        # output DMAs ----
        for blk in range(OT // 4):
            st = stg.tile([P, 4 * J], bf16, tag="stage", name=f"st{blk}")
            for pl in range(2):
                op = ops.tile([P, 2 * J], f32, tag="opsum", name=f"op{blk}_{pl}")
                for half in range(2):
                    o = blk * 4 + pl * 2 + half
                    g = o % 4
                    nc.tensor.matmul(op[:, half * J:(half + 1) * J],
                                     lhsT=vto4[32 * g:32 * g + R,
                                               o * P:(o + 1) * P],
                                     rhs=bsb4[32 * g:32 * g + R, :],
                                     start=True, stop=True,
                                     tile_position=(32 * g, 0))
                dst = st[:, pl * 2 * J:(pl + 1) * 2 * J]
                if pl % 2 == 0:
                    nc.scalar.activation(dst, op[:], Copy)
                else:
                    nc.vector.tensor_copy(dst, op[:])
            nc.sync.dma_start(out_ext[:, blk * 4 * J:(blk + 1) * 4 * J], st[:])

    nc.compile()
    return nc


_CACHE = {}


def _get_nc():
    if "nc" not in _CACHE:
        _CACHE["nc"] = build_bass()
    return _CACHE["nc"]


def _in_maps(W, b, freqs, afreqs):
    import ml_dtypes
    bf = ml_dtypes.bfloat16

    t, hw, U, chs, Ucos, chc = _factorization()
    V = _eval_chebs(chs, freqs)             # [D, R] float64
    U2 = U * hw[:, None]

    # shared constants
    vkt = V.reshape(KT, P, R).transpose(1, 0, 2).reshape(P, KT * R)
    u2pm = np.stack([U2.reshape(MT, P, R), -U2.reshape(MT, P, R)],
                    axis=1)                              # [MT, 2, P, R]
    u2t = u2pm.transpose(2, 0, 1, 3).reshape(P, MT * 2 * R)
    cstb = np.ascontiguousarray(
        np.concatenate([vkt, u2t], axis=1)).astype(bf)

    pack_base = np.zeros((KZ, PK_W), dtype=np.float64)
    pack_base[0:R, PK_UTZ:PK_UTZ + N] = U.T
    pack_base[R, PK_UTZ:PK_UTZ + N] = 1.0
    pack_base[0:R, PK_VTO:PK_VTO + D] = V.T
    pack_base[0:R, PK_UCO:PK_UCO + N] = Ucos.T

    vto4 = np.zeros((3 * 32 + R, D), dtype=np.float64)
    for g in range(4):
        vto4[32 * g:32 * g + R, :] = V.T
    vto4 = np.ascontiguousarray(vto4).astype(bf)

    maps = []
    for i in range(NCORES):
        sl = slice(i * J, (i + 1) * J)
        wkt = np.ascontiguousarray(
            W[:, sl].reshape(KT, P, J).transpose(1, 0, 2).reshape(P, KT * J)
        ).astype(bf)
        pk = pack_base.copy()
        pk[0:R, PK_VCA:PK_VCA + J] = _eval_chebs(chc, afreqs[sl]).T
        pk[R, PK_BRW:PK_BRW + J] = b[sl]
        maps.append({
            "wkt": wkt,
            "pack": np.ascontiguousarray(pk).astype(bf),
            "cstb": cstb,
            "vto4": vto4,
        })
    return maps


def _assemble(res):
    outs = []
    for i in range(NCORES):
        o = np.asarray(res.results[i]["out"]).astype(np.float32)
        outs.append(o.reshape(P, OT, J).transpose(1, 0, 2).reshape(D, J))
    return np.concatenate(outs, axis=1)


def kernel(W, b, freqs, afreqs):
    from concourse.bass_utils import run_bass_kernel_spmd

    W = np.ascontiguousarray(np.asarray(W, dtype=np.float32))
    b = np.asarray(b, dtype=np.float32)
    freqs = np.asarray(freqs, dtype=np.float32)
    afreqs = np.asarray(afreqs, dtype=np.float32)

    nc = _get_nc()
    maps = _in_maps(W, b, freqs, afreqs)
    res = run_bass_kernel_spmd(nc, maps, core_ids=list(range(NCORES)))
    return _assemble(res)
